# revision 1
# baseline (speedup 1.0000x reference)
"""Trainium2 Bass kernel for the pre-LN multi-head attention block.

Sharding: 8 cores = 4 batches x 2 query-row halves, collective-free. Each core
computes all 16 heads for its 512 query rows, with full-T k/v for its batch
(k/v compute duplicated across the 2 cores of a batch).

Per-core scheme (C=1024 channels, T=1024 rows, TQ=512 query rows):
  - x^T [C, T] is loaded directly (host transposes; query rows rotated first)
    and normalized in place to z. LN stats come from ones-vector matmuls
    (partition-dim reduction on the PE); rstd = exp(-0.5*ln(var+eps)) on ACT;
    mean/rstd rows are broadcast across partitions via a DRAM bounce.
  - q^T [C, TQ], k^T [C, T] = W^T z^T keep channels on partitions, so the
    qk-LN gains/biases are per-partition scalars (tensor_scalar); their LN
    stats are again ones-matmuls.
  - v [T, C] natural = z^T.T Wv, stored head-interleaved with a ones column
    every 65 cols (softmax denominator augmentation).
  - scores^T per head = matmul(lhsT=k-hat slice, rhs=q-hat slice); two heads
    per 128-channel chunk via row strips 0-63 / 64-127. exp on ACT with the
    1/8 softmax scale folded in; no max-subtraction (scores are O(1) after
    qk-LN of activations drawn from the reference distribution).
  - attn@v: even head = matmul(lhsT=v_aug [128,65]) at psum base 0 (row 64
    accumulates the denominator); odd head = matmul(lhsT=v [128,64]) writing
    at psum base 64 directly, denominator via a separate ones-matmul tile.
    All PSUM reads in this phase are on ACT (ScalarE and VectorE must not
    touch the same PSUM bank in parallel).
  - denominators are DMA-collected to DRAM, reciprocal'd as one [16, TQ]
    batch, broadcast back per chunk, and multiplied into out^T.
  - proj: y^T = Wp^T out^T + bias -> host transposes/scatters.
"""

from contextlib import ExitStack

import numpy as np

import concourse.bacc as bacc
import concourse.mybir as mybir
import concourse.tile as tile
from concourse.bass_utils import run_bass_kernel_spmd

F32 = mybir.dt.float32
AF = mybir.ActivationFunctionType
OP = mybir.AluOpType

B, T, C = 4, 1024, 1024
H, D = 16, 64
TQ = 512           # query rows per core
NCH = 8            # 128-row chunks of C (or T)
EPS = 1e-5

_CACHE = {}


def _build(stop="full"):
    nc = bacc.Bacc(None, target_bir_lowering=False, debug=False)

    xT_d = nc.declare_dram_parameter("xT", [C, T], F32, isOutput=False)
    wq_d = nc.declare_dram_parameter("wq", [C, C], F32, isOutput=False)
    wk_d = nc.declare_dram_parameter("wk", [C, C], F32, isOutput=False)
    wv_d = nc.declare_dram_parameter("wv", [C, C], F32, isOutput=False)
    wp_d = nc.declare_dram_parameter("wp", [C, C], F32, isOutput=False)
    bq_d = nc.declare_dram_parameter("bq", [C], F32, isOutput=False)
    bk_d = nc.declare_dram_parameter("bk", [C], F32, isOutput=False)
    bv_d = nc.declare_dram_parameter("bv", [C], F32, isOutput=False)
    bp_d = nc.declare_dram_parameter("bp", [C], F32, isOutput=False)
    qg_d = nc.declare_dram_parameter("qg", [C], F32, isOutput=False)
    qb_d = nc.declare_dram_parameter("qb", [C], F32, isOutput=False)
    kg_d = nc.declare_dram_parameter("kg", [C], F32, isOutput=False)
    kb_d = nc.declare_dram_parameter("kb", [C], F32, isOutput=False)
    yT_d = nc.declare_dram_parameter("yT", [C, TQ], F32, isOutput=True)

    with tile.TileContext(nc) as tc, ExitStack() as ctx:
        pool = tc.tile_pool
        const = ctx.enter_context(pool(name="const", bufs=1))
        qsbp = ctx.enter_context(pool(name="qsb", bufs=1))
        ksbp = ctx.enter_context(pool(name="ksb", bufs=1))
        vsbp = ctx.enter_context(pool(name="vsb", bufs=1))
        bcp = ctx.enter_context(pool(name="bc", bufs=2))
        rows1 = ctx.enter_context(pool(name="rows1", bufs=1))
        rows2 = ctx.enter_context(pool(name="rows2", bufs=2))
        sqp = ctx.enter_context(pool(name="sq", bufs=3))
        wmmp = ctx.enter_context(pool(name="wmm", bufs=3))
        dram = ctx.enter_context(pool(name="dram", bufs=1, space="DRAM"))

        # ---- constants ----
        def vec8(name, d):
            t = const.tile([128, 8], F32, tag=name)
            nc.sync.dma_start(out=t, in_=d.ap().rearrange("(j p) -> p j", p=128))
            return t

        bq8 = vec8("bq8", bq_d)
        bk8 = vec8("bk8", bk_d)
        bp8 = vec8("bp8", bp_d)
        qg8 = vec8("qg8", qg_d)
        qb8 = vec8("qb8", qb_d)
        kg8 = vec8("kg8", kg_d)
        kb8 = vec8("kb8", kb_d)
        ones1 = const.tile([128, 1], F32)
        nc.vector.memset(ones1, 1.0)
        eps1 = const.tile([1, 1], F32)
        nc.vector.memset(eps1, EPS)
        bvb = const.tile([128, C], F32)
        nc.sync.dma_start(out=bvb, in_=bv_d.ap().rearrange("c -> () c").to_broadcast([128, C]))

        # persistent activations
        q_sb = qsbp.tile([128, NCH, TQ], F32)      # q^T, later q-hat
        k_sb = ksbp.tile([128, NCH, T], F32)       # k^T, later k-hat
        v_sb = vsbp.tile([128, NCH, H * 65], F32)  # v head-interleaved + ones col

        v_ones_view = v_sb.rearrange("p i (h x) -> p i h x", x=65)[:, :, :, 64:65]
        nc.vector.memset(v_ones_view, 1.0)

        # DRAM scratch
        scr_x = dram.tile([1, 2 * T], F32)
        scr_q = dram.tile([1, 2 * TQ], F32)
        scr_k = dram.tile([1, 2 * T], F32)
        scr_den = dram.tile([H, TQ], F32)
        scr_rec = dram.tile([H, TQ], F32)

        def ln_rows(pack, srow, n, scr):
            """pack[:, 0:n] = mean, pack[:, n:2n] = rstd from raw [sum|sumsq]
            rows in srow; DMA pack to DRAM scratch scr."""
            mu = pack[:, 0:n]
            rs = pack[:, n:2 * n]
            nc.vector.tensor_scalar(out=mu, in0=srow[:, 0:n], scalar1=1.0 / C, scalar2=None, op0=OP.mult)
            ex2 = rows2.tile([1, T], F32, tag="rowtmp")
            nc.vector.tensor_scalar(out=ex2[:, 0:n], in0=srow[:, n:2 * n], scalar1=1.0 / C, scalar2=None, op0=OP.mult)
            musq = rows2.tile([1, T], F32, tag="rowtmp")
            nc.vector.tensor_tensor(out=musq[:, 0:n], in0=mu, in1=mu, op=OP.mult)
            nc.vector.tensor_tensor(out=ex2[:, 0:n], in0=ex2[:, 0:n], in1=musq[:, 0:n], op=OP.subtract)
            nc.scalar.activation(out=ex2[:, 0:n], in_=ex2[:, 0:n], func=AF.Ln, bias=eps1, scale=1.0)
            nc.scalar.activation(out=rs, in_=ex2[:, 0:n], func=AF.Exp, scale=-0.5)
            nc.sync.dma_start(out=scr[:, :], in_=pack)

        # ================= phase A: load x^T, stats, normalize =================
        xz_ctx = ExitStack()
        xzp = xz_ctx.enter_context(pool(name="xz", bufs=1))
        xT_sb = xzp.tile([128, NCH, T], F32)
        for j in range(NCH):
            nc.sync.dma_start(out=xT_sb[:, j, :], in_=xT_d[j * 128:(j + 1) * 128, :])

        psA_ctx = ExitStack()
        psA = psA_ctx.enter_context(pool(name="psA", bufs=1, space="PSUM"))
        xstat_ps = psA.tile([1, 2 * T], F32)
        for j in range(NCH):
            sqt = sqp.tile([128, T], F32, tag="sq")
            nc.vector.tensor_tensor(out=sqt, in0=xT_sb[:, j, :], in1=xT_sb[:, j, :], op=OP.mult)
            st, sp = j == 0, j == NCH - 1
            for n in range(2):
                nc.tensor.matmul(xstat_ps[0:1, n * 512:(n + 1) * 512], ones1,
                                 xT_sb[:, j, n * 512:(n + 1) * 512], start=st, stop=sp)
                nc.tensor.matmul(xstat_ps[0:1, T + n * 512:T + (n + 1) * 512], ones1,
                                 sqt[:, n * 512:(n + 1) * 512], start=st, stop=sp)
        srow = rows1.tile([1, 2 * T], F32, tag="srow")
        nc.vector.tensor_copy(out=srow, in_=xstat_ps)
        psA_ctx.close()

        xpack = rows1.tile([1, 2 * T], F32, tag="pack")
        ln_rows(xpack, srow, T, scr_x)
        mub = bcp.tile([128, T], F32, tag="bc")
        nc.sync.dma_start(out=mub, in_=scr_x[0:1, 0:T].to_broadcast([128, T]))
        rsb = bcp.tile([128, T], F32, tag="bc")
        nc.sync.dma_start(out=rsb, in_=scr_x[0:1, T:2 * T].to_broadcast([128, T]))
        for j in range(NCH):
            tz = sqp.tile([128, T], F32, tag="sq")
            nc.vector.tensor_tensor(out=tz, in0=xT_sb[:, j, :], in1=mub, op=OP.subtract)
            nc.vector.tensor_tensor(out=xT_sb[:, j, :], in0=tz, in1=rsb, op=OP.mult)

        # ================= phase B: q / k / v matmuls =================
        mm_ctx = ExitStack()
        mmp = mm_ctx.enter_context(pool(name="mm", bufs=2, space="PSUM"))
        wvp = mm_ctx.enter_context(pool(name="wvpool", bufs=2))

        def wslab(wd, m):
            t = wmmp.tile([128, NCH, 128], F32, tag="w")
            nc.sync.dma_start(out=t, in_=wd.ap().rearrange("(j p) c -> p j c", p=128)[:, :, m * 128:(m + 1) * 128])
            return t

        # --- q ---
        for m in range(NCH):
            wsl = wslab(wq_d, m)
            q_ps = mmp.tile([128, T], F32, tag="mm")
            for j in range(NCH):
                nc.tensor.matmul(q_ps[:, 0:TQ], wsl[:, j, :], xT_sb[:, j, 0:TQ],
                                 start=(j == 0), stop=(j == NCH - 1))
            nc.scalar.activation(out=q_sb[:, m, :], in_=q_ps[:, 0:TQ], func=AF.Identity,
                                 bias=bq8[:, m:m + 1], scale=1.0)
        qs_ctx = ExitStack()
        qstatp = qs_ctx.enter_context(pool(name="qstat", bufs=1, space="PSUM"))
        qstat_ps = qstatp.tile([1, 2 * TQ], F32)
        for m in range(NCH):
            sqt = sqp.tile([128, T], F32, tag="sq")
            nc.vector.tensor_tensor(out=sqt[:, 0:TQ], in0=q_sb[:, m, :], in1=q_sb[:, m, :], op=OP.mult)
            nc.tensor.matmul(qstat_ps[0:1, 0:TQ], ones1, q_sb[:, m, :],
                             start=(m == 0), stop=(m == NCH - 1))
            nc.tensor.matmul(qstat_ps[0:1, TQ:2 * TQ], ones1, sqt[:, 0:TQ],
                             start=(m == 0), stop=(m == NCH - 1))
        qsrow = rows1.tile([1, 2 * T], F32, tag="srow")
        nc.vector.tensor_copy(out=qsrow[:, 0:2 * TQ], in_=qstat_ps)
        qs_ctx.close()
        qpack = rows1.tile([1, 2 * T], F32, tag="pack")
        ln_rows(qpack[:, 0:2 * TQ], qsrow, TQ, scr_q)
        muqb = bcp.tile([128, T], F32, tag="bc")
        nc.sync.dma_start(out=muqb[:, 0:TQ], in_=scr_q[0:1, 0:TQ].to_broadcast([128, TQ]))
        rsqb = bcp.tile([128, T], F32, tag="bc")
        nc.sync.dma_start(out=rsqb[:, 0:TQ], in_=scr_q[0:1, TQ:2 * TQ].to_broadcast([128, TQ]))
        for m in range(NCH):
            t1 = sqp.tile([128, T], F32, tag="sq")
            nc.vector.tensor_tensor(out=t1[:, 0:TQ], in0=q_sb[:, m, :], in1=muqb[:, 0:TQ], op=OP.subtract)
            t2 = sqp.tile([128, T], F32, tag="sq")
            nc.vector.tensor_tensor(out=t2[:, 0:TQ], in0=t1[:, 0:TQ], in1=rsqb[:, 0:TQ], op=OP.mult)
            nc.vector.tensor_scalar(out=q_sb[:, m, :], in0=t2[:, 0:TQ],
                                    scalar1=qg8[:, m:m + 1], scalar2=qb8[:, m:m + 1],
                                    op0=OP.mult, op1=OP.add)

        # --- k ---
        for m in range(NCH):
            wsl = wslab(wk_d, m)
            k_ps = mmp.tile([128, T], F32, tag="mm")
            for n in range(2):
                for j in range(NCH):
                    nc.tensor.matmul(k_ps[:, n * 512:(n + 1) * 512], wsl[:, j, :],
                                     xT_sb[:, j, n * 512:(n + 1) * 512],
                                     start=(j == 0), stop=(j == NCH - 1))
            nc.scalar.activation(out=k_sb[:, m, :], in_=k_ps, func=AF.Identity,
                                 bias=bk8[:, m:m + 1], scale=1.0)
        ks_ctx = ExitStack()
        kstatp = ks_ctx.enter_context(pool(name="kstat", bufs=1, space="PSUM"))
        kstat_ps = kstatp.tile([1, 2 * T], F32)
        for m in range(NCH):
            sqt = sqp.tile([128, T], F32, tag="sq")
            nc.vector.tensor_tensor(out=sqt, in0=k_sb[:, m, :], in1=k_sb[:, m, :], op=OP.mult)
            for n in range(2):
                nc.tensor.matmul(kstat_ps[0:1, n * 512:(n + 1) * 512], ones1,
                                 k_sb[:, m, n * 512:(n + 1) * 512],
                                 start=(m == 0), stop=(m == NCH - 1))
                nc.tensor.matmul(kstat_ps[0:1, T + n * 512:T + (n + 1) * 512], ones1,
                                 sqt[:, n * 512:(n + 1) * 512],
                                 start=(m == 0), stop=(m == NCH - 1))
        ksrow = rows1.tile([1, 2 * T], F32, tag="srow")
        nc.vector.tensor_copy(out=ksrow, in_=kstat_ps)
        ks_ctx.close()
        kpack = rows1.tile([1, 2 * T], F32, tag="pack")
        ln_rows(kpack, ksrow, T, scr_k)
        mukb = bcp.tile([128, T], F32, tag="bc")
        nc.sync.dma_start(out=mukb, in_=scr_k[0:1, 0:T].to_broadcast([128, T]))
        rskb = bcp.tile([128, T], F32, tag="bc")
        nc.sync.dma_start(out=rskb, in_=scr_k[0:1, T:2 * T].to_broadcast([128, T]))
        for m in range(NCH):
            t1 = sqp.tile([128, T], F32, tag="sq")
            nc.vector.tensor_tensor(out=t1, in0=k_sb[:, m, :], in1=mukb, op=OP.subtract)
            t2 = sqp.tile([128, T], F32, tag="sq")
            nc.vector.tensor_tensor(out=t2, in0=t1, in1=rskb, op=OP.mult)
            nc.vector.tensor_scalar(out=k_sb[:, m, :], in0=t2,
                                    scalar1=kg8[:, m:m + 1], scalar2=kb8[:, m:m + 1],
                                    op0=OP.mult, op1=OP.add)

        # --- v ---
        for g in range(4):
            wvsl = wvp.tile([128, NCH, 256], F32, tag="wv")
            nc.sync.dma_start(
                out=wvsl, in_=wv_d.ap().rearrange("(j p) c -> p j c", p=128)[:, :, g * 256:(g + 1) * 256])
            for i in range(NCH):
                v_ps = mmp.tile([128, T], F32, tag="mm")
                for j in range(NCH):
                    nc.tensor.matmul(v_ps[:, 0:256], xT_sb[:, j, i * 128:(i + 1) * 128],
                                     wvsl[:, j, :], start=(j == 0), stop=(j == NCH - 1))
                vout = v_sb.rearrange("p i (h x) -> p i h x", x=65)[:, i, g * 4:(g + 1) * 4, 0:64]
                vin = v_ps[:, 0:256].rearrange("p (h x) -> p h x", x=64)
                nc.vector.tensor_tensor(
                    out=vout, in0=vin,
                    in1=bvb[:, g * 256:(g + 1) * 256].rearrange("p (h x) -> p h x", x=64),
                    op=OP.add)

        mm_ctx.close()
        xz_ctx.close()

        # ================= phase C: attention =================
        osbp = ctx.enter_context(pool(name="osb", bufs=1))
        outT_sb = osbp.tile([128, NCH, TQ], F32)
        pexpp = ctx.enter_context(pool(name="pexp", bufs=4))
        denp = ctx.enter_context(pool(name="den", bufs=3))
        rcbp = ctx.enter_context(pool(name="rcb", bufs=4))
        scp = ctx.enter_context(pool(name="sc", bufs=2, space="PSUM"))
        avpp0 = ctx.enter_context(pool(name="avp0", bufs=1, space="PSUM"))
        avpp1 = ctx.enter_context(pool(name="avp1", bufs=1, space="PSUM"))
        pjp = ctx.enter_context(pool(name="pj", bufs=1, space="PSUM"))
        youtp = ctx.enter_context(pool(name="yout", bufs=2))

        p_tiles = {}

        def emit_scores(m):
            p_list = []
            for i in range(NCH):
                sc_ps = scp.tile([128, 1024], F32, tag="sc")
                nc.tensor.matmul(sc_ps[:, 0:512], k_sb[0:64, m, i * 128:(i + 1) * 128],
                                 q_sb[0:64, m, :], start=True, stop=True)
                nc.tensor.matmul(sc_ps[:, 512:1024], k_sb[64:128, m, i * 128:(i + 1) * 128],
                                 q_sb[64:128, m, :], start=True, stop=True)
                p_sb = pexpp.tile([128, 1024], F32, tag="p")
                nc.scalar.activation(out=p_sb, in_=sc_ps[:, 0:1024], func=AF.Exp, scale=0.125)
                p_list.append(p_sb)
            p_tiles[m] = p_list

        def emit_av(m):
            p_list = p_tiles.pop(m)
            h0, h1 = 2 * m, 2 * m + 1
            av0 = avpp0.tile([65, TQ], F32, tag="av0")
            av1 = avpp1.tile([128, TQ], F32, tag="av1")
            dn1 = avpp0.tile([1, TQ], F32, tag="dn1")
            for i in range(NCH):
                st, sp = i == 0, i == NCH - 1
                nc.tensor.matmul(av0, v_sb[:, i, h0 * 65:h0 * 65 + 65],
                                 p_list[i][:, 0:512], start=st, stop=sp)
                nc.tensor.matmul(av1[64:128, :], v_sb[:, i, h1 * 65:h1 * 65 + 64],
                                 p_list[i][:, 512:1024], start=st, stop=sp)
                nc.tensor.matmul(dn1, ones1, p_list[i][:, 512:1024], start=st, stop=sp)
            nc.scalar.activation(out=outT_sb[0:64, m, :], in_=av0[0:64, :],
                                 func=AF.Identity, bias=0.0, scale=1.0)
            nc.scalar.activation(out=outT_sb[64:128, m, :], in_=av1[64:128, :],
                                 func=AF.Identity, bias=0.0, scale=1.0)
            dent = denp.tile([65, TQ], F32, tag="den")
            nc.scalar.activation(out=dent[64:65, :], in_=av0[64:65, :],
                                 func=AF.Identity, bias=0.0, scale=1.0)
            nc.sync.dma_start(out=scr_den[h0:h0 + 1, :], in_=dent[64:65, :])
            dent2 = denp.tile([1, TQ], F32, tag="den2")
            nc.scalar.activation(out=dent2, in_=dn1, func=AF.Identity, bias=0.0, scale=1.0)
            nc.sync.dma_start(out=scr_den[h1:h1 + 1, :], in_=dent2)

        emit_scores(0)
        for m in range(NCH):
            if m + 1 < NCH:
                emit_scores(m + 1)
            emit_av(m)

        den16 = rows2.tile([H, TQ], F32, tag="d16")
        nc.sync.dma_start(out=den16, in_=scr_den[:, :])
        rec16 = rows2.tile([H, TQ], F32, tag="d16")
        nc.vector.reciprocal(out=rec16, in_=den16)
        nc.sync.dma_start(out=scr_rec[:, :], in_=rec16)
        for m in range(NCH):
            rb = rcbp.tile([128, TQ], F32, tag="rcb")
            nc.sync.dma_start(out=rb[0:64, :], in_=scr_rec[2 * m:2 * m + 1, :].to_broadcast([64, TQ]))
            nc.sync.dma_start(out=rb[64:128, :], in_=scr_rec[2 * m + 1:2 * m + 2, :].to_broadcast([64, TQ]))
            sl = outT_sb[:, m, :]
            nc.vector.tensor_tensor(out=sl, in0=sl, in1=rb, op=OP.mult)

        if stop == "attn":
            for m in range(NCH):
                t = youtp.tile([128, TQ], F32, tag="y")
                nc.vector.tensor_copy(out=t, in_=outT_sb[:, m, :])
                nc.sync.dma_start(out=yT_d[m * 128:(m + 1) * 128, :], in_=t)

        # ================= phase D: proj =================
        if stop == "full":
            for m in range(NCH):
                wsl = wslab(wp_d, m)
                y_ps = pjp.tile([128, TQ], F32, tag="pj")
                for j in range(NCH):
                    nc.tensor.matmul(y_ps, wsl[:, j, :], outT_sb[:, j, :],
                                     start=(j == 0), stop=(j == NCH - 1))
                y_sb = youtp.tile([128, TQ], F32, tag="y")
                nc.scalar.activation(out=y_sb, in_=y_ps, func=AF.Identity,
                                     bias=bp8[:, m:m + 1], scale=1.0)
                nc.sync.dma_start(out=yT_d[m * 128:(m + 1) * 128, :], in_=y_sb)

    nc.finalize()
    return nc


def _get_nc():
    if "nc" not in _CACHE:
        _CACHE["nc"] = _build()
    return _CACHE["nc"]


def _prep_inputs(x, norm_g, norm_b, qkv_w, qkv_b, qln_g, qln_b, kln_g, kln_b, proj_w, proj_b):
    x = np.asarray(x, dtype=np.float32)
    norm_g = np.asarray(norm_g, dtype=np.float32)
    norm_b = np.asarray(norm_b, dtype=np.float32)
    qkv_w = np.asarray(qkv_w, dtype=np.float32)
    qkv_b = np.asarray(qkv_b, dtype=np.float32)

    wfold = norm_g[:, None] * qkv_w                    # [C, 3C]
    bfold = qkv_b + norm_b @ qkv_w                     # [3C]
    wq = np.ascontiguousarray(wfold[:, 0:C])
    wk = np.ascontiguousarray(wfold[:, C:2 * C])
    wv = np.ascontiguousarray(wfold[:, 2 * C:3 * C])
    bq, bk, bv = bfold[0:C].copy(), bfold[C:2 * C].copy(), bfold[2 * C:3 * C].copy()

    common = dict(
        wq=wq, wk=wk, wv=wv,
        wp=np.ascontiguousarray(np.asarray(proj_w, dtype=np.float32)),
        bq=bq, bk=bk, bv=bv,
        bp=np.asarray(proj_b, dtype=np.float32).copy(),
        qg=np.asarray(qln_g, dtype=np.float32).copy(),
        qb=np.asarray(qln_b, dtype=np.float32).copy(),
        kg=np.asarray(kln_g, dtype=np.float32).copy(),
        kb=np.asarray(kln_b, dtype=np.float32).copy(),
    )
    in_maps = []
    for core in range(8):
        b, half = core // 2, core % 2
        xp = np.concatenate([x[b, TQ * half:], x[b, :TQ * half]], axis=0) if half else x[b]
        xT = np.ascontiguousarray(xp.T)
        in_maps.append(dict(common, xT=xT))
    return in_maps


def kernel(**inputs) -> np.ndarray:
    in_maps = _prep_inputs(**inputs)
    nc = _get_nc()
    res = run_bass_kernel_spmd(nc, in_maps, core_ids=list(range(8)))
    out = np.empty((B, T, C), dtype=np.float32)
    for core in range(8):
        b, half = core // 2, core % 2
        out[b, TQ * half:TQ * half + TQ, :] = res.results[core]["yT"].T
    return out



# revision 25
# speedup vs baseline: 2.2630x; 2.2630x over previous
"""Trainium2 Bass kernel for the pre-LN multi-head attention block.

Sharding: 8 cores = 4 batches x 2 query-row halves, collective-free. Each core
computes all 16 heads for its 512 query rows, with full-T k/v for its batch
(k/v compute duplicated across the 2 cores of a batch).

Per-core scheme (C=1024 channels, T=1024 rows, TQ=512 query rows):
  - x^T [C, T] is loaded directly (host transposes; query rows rotated first)
    and normalized in place to z. LN stats come from ones-vector matmuls
    (partition-dim reduction on the PE); rstd = exp(-0.5*ln(var+eps)) on ACT;
    mean/rstd rows are broadcast across partitions via a DRAM bounce.
  - q^T [C, TQ], k^T [C, T] = W^T z^T keep channels on partitions, so the
    qk-LN gains/biases are per-partition scalars (tensor_scalar); their LN
    stats are again ones-matmuls.
  - v [T, C] natural = z^T.T Wv, stored head-interleaved with a ones column
    every 65 cols (softmax denominator augmentation).
  - scores^T per head = matmul(lhsT=k-hat slice, rhs=q-hat slice); two heads
    per 128-channel chunk via row strips 0-63 / 64-127. exp on ACT with the
    1/8 softmax scale folded in; no max-subtraction (scores are O(1) after
    qk-LN of activations drawn from the reference distribution).
  - attn@v: even head = matmul(lhsT=v_aug [128,65]) at psum base 0 (row 64
    accumulates the denominator); odd head = matmul(lhsT=v [128,64]) writing
    at psum base 64 directly, denominator via a separate ones-matmul tile.
    All PSUM reads in this phase are on ACT (ScalarE and VectorE must not
    touch the same PSUM bank in parallel).
  - denominators are DMA-collected to DRAM, reciprocal'd as one [16, TQ]
    batch, broadcast back per chunk, and multiplied into out^T.
  - proj: y^T = Wp^T out^T + bias -> host transposes/scatters.
"""

from contextlib import ExitStack

import numpy as np

import concourse.bacc as bacc
import concourse.mybir as mybir
import concourse.tile as tile
from concourse.bass_utils import run_bass_kernel_spmd

F32 = mybir.dt.float32
F32R = mybir.dt.float32r
AF = mybir.ActivationFunctionType
OP = mybir.AluOpType

B, T, C = 4, 1024, 1024
H, D = 16, 64
TQ = 512           # query rows per core
NCH = 8            # 128-row chunks of C (or T)
EPS = 1e-5

_CACHE = {}


def _build(stop="full"):
    nc = bacc.Bacc(None, target_bir_lowering=False, debug=False)

    xT_d = nc.declare_dram_parameter("xT", [C, T], F32, isOutput=False)
    wq_d = nc.declare_dram_parameter("wq", [C, C], F32, isOutput=False)
    wk_d = nc.declare_dram_parameter("wk", [C, C], F32, isOutput=False)
    wv_d = nc.declare_dram_parameter("wv", [C, C], F32, isOutput=False)
    wp_d = nc.declare_dram_parameter("wp", [C, C], F32, isOutput=False)
    bq_d = nc.declare_dram_parameter("bq", [C], F32, isOutput=False)
    bk_d = nc.declare_dram_parameter("bk", [C], F32, isOutput=False)
    bv_d = nc.declare_dram_parameter("bv", [C], F32, isOutput=False)
    bp_d = nc.declare_dram_parameter("bp", [C], F32, isOutput=False)
    qg_d = nc.declare_dram_parameter("qg", [C], F32, isOutput=False)
    qb_d = nc.declare_dram_parameter("qb", [C], F32, isOutput=False)
    kg_d = nc.declare_dram_parameter("kg", [C], F32, isOutput=False)
    kb_d = nc.declare_dram_parameter("kb", [C], F32, isOutput=False)
    yT_d = nc.declare_dram_parameter("yT", [C, TQ], F32, isOutput=True)

    with tile.TileContext(nc) as tc, ExitStack() as ctx:
        pool = tc.tile_pool

        def mmr(out, lhsT, rhs, **kw):
            # float32r: single-pass reduced-precision fp32 matmul (1 cyc/row
            # at free dim >= 256, vs 4 for fp32)
            nc.tensor.matmul(out, lhsT.bitcast(F32R), rhs.bitcast(F32R), **kw)
        const = ctx.enter_context(pool(name="const", bufs=1))
        qsbp = ctx.enter_context(pool(name="qsb", bufs=1))
        ksbp = ctx.enter_context(pool(name="ksb", bufs=1))
        vsbp = ctx.enter_context(pool(name="vsb", bufs=1))
        bcp = ctx.enter_context(pool(name="bc", bufs=2))
        rows1 = ctx.enter_context(pool(name="rows1", bufs=1))
        rows2 = ctx.enter_context(pool(name="rows2", bufs=2))
        sqp = ctx.enter_context(pool(name="sq", bufs=3))
        wmmp = ctx.enter_context(pool(name="wmm", bufs=3))
        dram = ctx.enter_context(pool(name="dram", bufs=1, space="DRAM"))

        # ---- constants ----
        def vec8(name, d):
            t = const.tile([128, 8], F32, tag=name)
            nc.sync.dma_start(out=t, in_=d.ap().rearrange("(j p) -> p j", p=128))
            return t

        bq8 = vec8("bq8", bq_d)
        bk8 = vec8("bk8", bk_d)
        bp8 = vec8("bp8", bp_d)
        qg8 = vec8("qg8", qg_d)
        qb8 = vec8("qb8", qb_d)
        kg8 = vec8("kg8", kg_d)
        kb8 = vec8("kb8", kb_d)
        ones_blk = const.tile([128, 128], F32, tag="onesblk")
        nc.vector.memset(ones_blk, 1.0)
        ones1 = const.tile([128, 1], F32)
        nc.vector.tensor_copy(out=ones1.bitcast(F32R), in_=ones_blk[:, 0:1])
        eps1 = const.tile([1, 1], F32)
        nc.vector.memset(eps1, EPS)
        bvb = const.tile([128, C], F32)
        nc.sync.dma_start(out=bvb, in_=bv_d.ap().rearrange("c -> () c").to_broadcast([128, C]))

        # persistent activations
        q_sb = qsbp.tile([128, NCH, TQ], F32)      # q^T, later q-hat
        k_sb = ksbp.tile([128, NCH, T], F32)       # k^T, later k-hat
        v_sb = vsbp.tile([128, NCH, H * 65], F32)  # v head-interleaved + ones col

        v_ones_view = v_sb.rearrange("p i (h x) -> p i h x", x=65)[:, :, :, 64:65]
        nc.vector.tensor_copy(out=v_ones_view.bitcast(F32R),
                              in_=ones_blk.rearrange("p (i h x) -> p i h x", i=NCH, h=H))

        # DRAM scratch
        scr_x = dram.tile([1, 2 * T], F32)
        scr_q = dram.tile([1, 2 * TQ], F32)
        scr_k = dram.tile([1, 2 * T], F32)
        scr_den = dram.tile([H, TQ], F32)
        scr_rec = dram.tile([H, TQ], F32)

        def ln_rows(pack, srow, n, scr):
            """pack[:, 0:n] = mean, pack[:, n:2n] = rstd from raw [sum|sumsq]
            rows in srow; DMA pack to DRAM scratch scr."""
            mu = pack[:, 0:n]
            rs = pack[:, n:2 * n]
            nc.vector.tensor_scalar(out=mu, in0=srow[:, 0:n], scalar1=1.0 / C, scalar2=None, op0=OP.mult)
            ex2 = rows2.tile([1, T], F32, tag="rowtmp")
            nc.vector.tensor_scalar(out=ex2[:, 0:n], in0=srow[:, n:2 * n], scalar1=1.0 / C, scalar2=None, op0=OP.mult)
            musq = rows2.tile([1, T], F32, tag="rowtmp")
            nc.vector.tensor_tensor(out=musq[:, 0:n], in0=mu, in1=mu, op=OP.mult)
            nc.vector.tensor_tensor(out=ex2[:, 0:n], in0=ex2[:, 0:n], in1=musq[:, 0:n], op=OP.subtract)
            nc.scalar.activation(out=ex2[:, 0:n], in_=ex2[:, 0:n], func=AF.Ln, bias=eps1, scale=1.0)
            nc.scalar.activation(out=rs, in_=ex2[:, 0:n], func=AF.Exp, scale=-0.5)
            nc.sync.dma_start(out=scr[:, :], in_=pack)

        # ================= phase A: load x^T, stats, normalize =================
        xz_ctx = ExitStack()
        xzp = xz_ctx.enter_context(pool(name="xz", bufs=1))
        xT_sb = xzp.tile([128, NCH, T], F32)
        for j in range(NCH):
            nc.sync.dma_start(out=xT_sb[:, j, :].bitcast(F32R),
                              in_=xT_d[j * 128:(j + 1) * 128, :].bitcast(F32R))

        psA_ctx = ExitStack()
        psA = psA_ctx.enter_context(pool(name="psA", bufs=1, space="PSUM"))
        xstat_ps = psA.tile([1, 2 * T], F32)
        for j in range(NCH):
            sqt = sqp.tile([128, T], F32, tag="sq")
            nc.vector.tensor_tensor(out=sqt.bitcast(F32R), in0=xT_sb[:, j, :], in1=xT_sb[:, j, :], op=OP.mult)
            st, sp = j == 0, j == NCH - 1
            for n in range(2):
                mmr(xstat_ps[0:1, n * 512:(n + 1) * 512], ones1,
                                 xT_sb[:, j, n * 512:(n + 1) * 512], start=st, stop=sp)
                mmr(xstat_ps[0:1, T + n * 512:T + (n + 1) * 512], ones1,
                                 sqt[:, n * 512:(n + 1) * 512], start=st, stop=sp)
        srow = rows1.tile([1, 2 * T], F32, tag="srow")
        nc.vector.tensor_copy(out=srow, in_=xstat_ps)
        psA_ctx.close()

        xpack = rows1.tile([1, 2 * T], F32, tag="pack")
        ln_rows(xpack, srow, T, scr_x)
        mub = bcp.tile([128, T], F32, tag="bc")
        nc.sync.dma_start(out=mub, in_=scr_x[0:1, 0:T].to_broadcast([128, T]))
        rsb = bcp.tile([128, T], F32, tag="bc")
        nc.sync.dma_start(out=rsb, in_=scr_x[0:1, T:2 * T].to_broadcast([128, T]))
        for j in range(NCH):
            tz = sqp.tile([128, T], F32, tag="sq")
            nc.vector.tensor_tensor(out=tz, in0=xT_sb[:, j, :], in1=mub, op=OP.subtract)
            nc.vector.tensor_tensor(out=xT_sb[:, j, :].bitcast(F32R), in0=tz, in1=rsb, op=OP.mult)

        # ================= phase B: q / k / v matmuls =================
        mm_ctx = ExitStack()
        mmp = mm_ctx.enter_context(pool(name="mm", bufs=2, space="PSUM"))
        wvp = mm_ctx.enter_context(pool(name="wvpool", bufs=2))

        def wslab(wd, m):
            t = wmmp.tile([128, NCH, 128], F32, tag="w")
            nc.sync.dma_start(out=t.bitcast(F32R),
                              in_=wd.ap().rearrange("(j p) c -> p j c", p=128)[:, :, m * 128:(m + 1) * 128].bitcast(F32R))
            return t

        # --- q ---
        for m in range(NCH):
            wsl = wslab(wq_d, m)
            q_ps = mmp.tile([128, T], F32, tag="mm")
            for j in range(NCH):
                mmr(q_ps[:, 0:TQ], wsl[:, j, :], xT_sb[:, j, 0:TQ],
                                 start=(j == 0), stop=(j == NCH - 1))
            nc.scalar.activation(out=q_sb[:, m, :].bitcast(F32R), in_=q_ps[:, 0:TQ], func=AF.Identity,
                                 bias=bq8[:, m:m + 1], scale=1.0)
        qs_ctx = ExitStack()
        qstatp = qs_ctx.enter_context(pool(name="qstat", bufs=1, space="PSUM"))
        qstat_ps = qstatp.tile([1, 2 * TQ], F32)
        for m in range(NCH):
            sqt = sqp.tile([128, T], F32, tag="sq")
            nc.vector.tensor_tensor(out=sqt[:, 0:TQ].bitcast(F32R), in0=q_sb[:, m, :], in1=q_sb[:, m, :], op=OP.mult)
            mmr(qstat_ps[0:1, 0:TQ], ones1, q_sb[:, m, :],
                             start=(m == 0), stop=(m == NCH - 1))
            mmr(qstat_ps[0:1, TQ:2 * TQ], ones1, sqt[:, 0:TQ],
                             start=(m == 0), stop=(m == NCH - 1))
        qsrow = rows1.tile([1, 2 * T], F32, tag="srow")
        nc.vector.tensor_copy(out=qsrow[:, 0:2 * TQ], in_=qstat_ps)
        qs_ctx.close()
        qpack = rows1.tile([1, 2 * T], F32, tag="pack")
        ln_rows(qpack[:, 0:2 * TQ], qsrow, TQ, scr_q)
        muqb = bcp.tile([128, T], F32, tag="bc")
        nc.sync.dma_start(out=muqb[:, 0:TQ], in_=scr_q[0:1, 0:TQ].to_broadcast([128, TQ]))
        rsqb = bcp.tile([128, T], F32, tag="bc")
        nc.sync.dma_start(out=rsqb[:, 0:TQ], in_=scr_q[0:1, TQ:2 * TQ].to_broadcast([128, TQ]))
        for m in range(NCH):
            t1 = sqp.tile([128, T], F32, tag="sq")
            nc.vector.tensor_tensor(out=t1[:, 0:TQ], in0=q_sb[:, m, :], in1=muqb[:, 0:TQ], op=OP.subtract)
            t2 = sqp.tile([128, T], F32, tag="sq")
            nc.vector.tensor_tensor(out=t2[:, 0:TQ], in0=t1[:, 0:TQ], in1=rsqb[:, 0:TQ], op=OP.mult)
            nc.scalar.activation(out=q_sb[:, m, :].bitcast(F32R), in_=t2[:, 0:TQ],
                                 func=AF.Identity, bias=qb8[:, m:m + 1],
                                 scale=qg8[:, m:m + 1])

        # --- k ---
        for m in range(NCH):
            wsl = wslab(wk_d, m)
            k_ps = mmp.tile([128, T], F32, tag="mm")
            for n in range(2):
                for j in range(NCH):
                    mmr(k_ps[:, n * 512:(n + 1) * 512], wsl[:, j, :],
                                     xT_sb[:, j, n * 512:(n + 1) * 512],
                                     start=(j == 0), stop=(j == NCH - 1))
            nc.scalar.activation(out=k_sb[:, m, :].bitcast(F32R), in_=k_ps, func=AF.Identity,
                                 bias=bk8[:, m:m + 1], scale=1.0)
        ks_ctx = ExitStack()
        kstatp = ks_ctx.enter_context(pool(name="kstat", bufs=1, space="PSUM"))
        kstat_ps = kstatp.tile([1, 2 * T], F32)
        for m in range(NCH):
            sqt = sqp.tile([128, T], F32, tag="sq")
            nc.vector.tensor_tensor(out=sqt.bitcast(F32R), in0=k_sb[:, m, :], in1=k_sb[:, m, :], op=OP.mult)
            for n in range(2):
                mmr(kstat_ps[0:1, n * 512:(n + 1) * 512], ones1,
                                 k_sb[:, m, n * 512:(n + 1) * 512],
                                 start=(m == 0), stop=(m == NCH - 1))
                mmr(kstat_ps[0:1, T + n * 512:T + (n + 1) * 512], ones1,
                                 sqt[:, n * 512:(n + 1) * 512],
                                 start=(m == 0), stop=(m == NCH - 1))
        ksrow = rows1.tile([1, 2 * T], F32, tag="srow")
        nc.vector.tensor_copy(out=ksrow, in_=kstat_ps)
        ks_ctx.close()
        kpack = rows1.tile([1, 2 * T], F32, tag="pack")
        ln_rows(kpack, ksrow, T, scr_k)
        mukb = bcp.tile([128, T], F32, tag="bc")
        nc.sync.dma_start(out=mukb, in_=scr_k[0:1, 0:T].to_broadcast([128, T]))
        rskb = bcp.tile([128, T], F32, tag="bc")
        nc.sync.dma_start(out=rskb, in_=scr_k[0:1, T:2 * T].to_broadcast([128, T]))
        for m in range(NCH):
            t1 = sqp.tile([128, T], F32, tag="sq")
            nc.vector.tensor_tensor(out=t1, in0=k_sb[:, m, :], in1=mukb, op=OP.subtract)
            t2 = sqp.tile([128, T], F32, tag="sq")
            nc.vector.tensor_tensor(out=t2, in0=t1, in1=rskb, op=OP.mult)
            nc.scalar.activation(out=k_sb[:, m, :].bitcast(F32R), in_=t2,
                                 func=AF.Identity, bias=kb8[:, m:m + 1],
                                 scale=kg8[:, m:m + 1])

        # --- v ---
        for g in range(4):
            wvsl = wvp.tile([128, NCH, 256], F32, tag="wv")
            nc.sync.dma_start(
                out=wvsl.bitcast(F32R),
                in_=wv_d.ap().rearrange("(j p) c -> p j c", p=128)[:, :, g * 256:(g + 1) * 256].bitcast(F32R))
            for i in range(NCH):
                v_ps = mmp.tile([128, T], F32, tag="mm")
                for j in range(NCH):
                    mmr(v_ps[:, 0:256], xT_sb[:, j, i * 128:(i + 1) * 128],
                                     wvsl[:, j, :], start=(j == 0), stop=(j == NCH - 1))
                vout = v_sb.rearrange("p i (h x) -> p i h x", x=65)[:, i, g * 4:(g + 1) * 4, 0:64]
                vin = v_ps[:, 0:256].rearrange("p (h x) -> p h x", x=64)
                nc.vector.tensor_tensor(
                    out=vout.bitcast(F32R), in0=vin,
                    in1=bvb[:, g * 256:(g + 1) * 256].rearrange("p (h x) -> p h x", x=64),
                    op=OP.add)

        mm_ctx.close()
        xz_ctx.close()

        # ================= phase C: attention =================
        osbp = ctx.enter_context(pool(name="osb", bufs=1))
        outT_sb = osbp.tile([128, NCH, TQ], F32)
        pexpp = ctx.enter_context(pool(name="pexp", bufs=4))
        denp = ctx.enter_context(pool(name="den", bufs=3))
        rcbp = ctx.enter_context(pool(name="rcb", bufs=4))
        scp = ctx.enter_context(pool(name="sc", bufs=2, space="PSUM"))
        avpp0 = ctx.enter_context(pool(name="avp0", bufs=1, space="PSUM"))
        avpp1 = ctx.enter_context(pool(name="avp1", bufs=1, space="PSUM"))
        pjp = ctx.enter_context(pool(name="pj", bufs=1, space="PSUM"))
        youtp = ctx.enter_context(pool(name="yout", bufs=2))

        p_tiles = {}

        def emit_scores(m):
            p_list = []
            for i in range(NCH):
                sc_ps = scp.tile([128, 1024], F32, tag="sc")
                mmr(sc_ps[:, 0:512], k_sb[0:64, m, i * 128:(i + 1) * 128],
                                 q_sb[0:64, m, :], start=True, stop=True)
                mmr(sc_ps[:, 512:1024], k_sb[64:128, m, i * 128:(i + 1) * 128],
                                 q_sb[64:128, m, :], start=True, stop=True)
                p_sb = pexpp.tile([128, 1024], F32, tag="p")
                nc.scalar.activation(out=p_sb.bitcast(F32R), in_=sc_ps[:, 0:1024], func=AF.Exp, scale=0.125)
                p_list.append(p_sb)
            p_tiles[m] = p_list

        def emit_av(m):
            p_list = p_tiles.pop(m)
            h0, h1 = 2 * m, 2 * m + 1
            av0 = avpp0.tile([65, TQ], F32, tag="av0")
            av1 = avpp1.tile([65, TQ], F32, tag="av1")
            for i in range(NCH):
                st, sp = i == 0, i == NCH - 1
                mmr(av0, v_sb[:, i, h0 * 65:h0 * 65 + 65],
                                 p_list[i][:, 0:512], start=st, stop=sp)
                mmr(av1, v_sb[:, i, h1 * 65:h1 * 65 + 65],
                                 p_list[i][:, 512:1024], start=st, stop=sp)
            nc.scalar.activation(out=outT_sb[0:64, m, :].bitcast(F32R), in_=av0[0:64, :],
                                 func=AF.Identity, bias=0.0, scale=1.0)
            nc.scalar.activation(out=outT_sb[64:128, m, :].bitcast(F32R), in_=av1[0:64, :],
                                 func=AF.Identity, bias=0.0, scale=1.0)
            dent = denp.tile([65, TQ], F32, tag="den")
            nc.scalar.activation(out=dent[64:65, :], in_=av0[64:65, :],
                                 func=AF.Identity, bias=0.0, scale=1.0)
            nc.sync.dma_start(out=scr_den[h0:h0 + 1, :], in_=dent[64:65, :])
            dent2 = denp.tile([65, TQ], F32, tag="den2")
            nc.scalar.activation(out=dent2[64:65, :], in_=av1[64:65, :],
                                 func=AF.Identity, bias=0.0, scale=1.0)
            nc.sync.dma_start(out=scr_den[h1:h1 + 1, :], in_=dent2[64:65, :])

        emit_scores(0)
        for m in range(NCH):
            if m + 1 < NCH:
                emit_scores(m + 1)
            emit_av(m)

        den16 = rows2.tile([H, TQ], F32, tag="d16")
        nc.sync.dma_start(out=den16, in_=scr_den[:, :])
        rec16 = rows2.tile([H, TQ], F32, tag="d16")
        nc.vector.reciprocal(out=rec16, in_=den16)
        nc.sync.dma_start(out=scr_rec[:, :], in_=rec16)
        for m in range(NCH):
            rb = rcbp.tile([128, TQ], F32, tag="rcb")
            nc.sync.dma_start(out=rb[0:64, :], in_=scr_rec[2 * m:2 * m + 1, :].to_broadcast([64, TQ]))
            nc.sync.dma_start(out=rb[64:128, :], in_=scr_rec[2 * m + 1:2 * m + 2, :].to_broadcast([64, TQ]))
            sl = outT_sb[:, m, :]
            nc.vector.tensor_tensor(out=sl.bitcast(F32R), in0=sl, in1=rb, op=OP.mult)

        if stop == "attn":
            for m in range(NCH):
                t = youtp.tile([128, TQ], F32, tag="y")
                nc.vector.tensor_copy(out=t, in_=outT_sb[:, m, :])
                nc.sync.dma_start(out=yT_d[m * 128:(m + 1) * 128, :], in_=t)

        # ================= phase D: proj =================
        if stop == "full":
            for m in range(NCH):
                wsl = wslab(wp_d, m)
                y_ps = pjp.tile([128, TQ], F32, tag="pj")
                for j in range(NCH):
                    mmr(y_ps, wsl[:, j, :], outT_sb[:, j, :],
                                     start=(j == 0), stop=(j == NCH - 1))
                y_sb = youtp.tile([128, TQ], F32, tag="y")
                nc.scalar.activation(out=y_sb, in_=y_ps, func=AF.Identity,
                                     bias=bp8[:, m:m + 1], scale=1.0)
                nc.sync.dma_start(out=yT_d[m * 128:(m + 1) * 128, :], in_=y_sb)

    nc.finalize()
    return nc


def _get_nc():
    if "nc" not in _CACHE:
        _CACHE["nc"] = _build()
    return _CACHE["nc"]


def _prep_inputs(x, norm_g, norm_b, qkv_w, qkv_b, qln_g, qln_b, kln_g, kln_b, proj_w, proj_b):
    x = np.asarray(x, dtype=np.float32)
    norm_g = np.asarray(norm_g, dtype=np.float32)
    norm_b = np.asarray(norm_b, dtype=np.float32)
    qkv_w = np.asarray(qkv_w, dtype=np.float32)
    qkv_b = np.asarray(qkv_b, dtype=np.float32)

    wfold = norm_g[:, None] * qkv_w                    # [C, 3C]
    bfold = qkv_b + norm_b @ qkv_w                     # [3C]
    wq = np.ascontiguousarray(wfold[:, 0:C])
    wk = np.ascontiguousarray(wfold[:, C:2 * C])
    wv = np.ascontiguousarray(wfold[:, 2 * C:3 * C])
    bq, bk, bv = bfold[0:C].copy(), bfold[C:2 * C].copy(), bfold[2 * C:3 * C].copy()

    common = dict(
        wq=wq, wk=wk, wv=wv,
        wp=np.ascontiguousarray(np.asarray(proj_w, dtype=np.float32)),
        bq=bq, bk=bk, bv=bv,
        bp=np.asarray(proj_b, dtype=np.float32).copy(),
        qg=np.asarray(qln_g, dtype=np.float32).copy(),
        qb=np.asarray(qln_b, dtype=np.float32).copy(),
        kg=np.asarray(kln_g, dtype=np.float32).copy(),
        kb=np.asarray(kln_b, dtype=np.float32).copy(),
    )
    in_maps = []
    for core in range(8):
        b, half = core // 2, core % 2
        xp = np.concatenate([x[b, TQ * half:], x[b, :TQ * half]], axis=0) if half else x[b]
        xT = np.ascontiguousarray(xp.T)
        in_maps.append(dict(common, xT=xT))
    return in_maps


def kernel(**inputs) -> np.ndarray:
    in_maps = _prep_inputs(**inputs)
    nc = _get_nc()
    res = run_bass_kernel_spmd(nc, in_maps, core_ids=list(range(8)))
    out = np.empty((B, T, C), dtype=np.float32)
    for core in range(8):
        b, half = core // 2, core % 2
        out[b, TQ * half:TQ * half + TQ, :] = res.results[core]["yT"].T
    return out

